# revision 1
# baseline (speedup 1.0000x reference)
"""Trainium2 Bass kernel for nn_LA_283467842715.

Math (per batch b, head h of 16, each head owning 128 contiguous channels):
  means/maxs over (128 group channels x 2x2 patch) -> [B,16,4,4]
  tiny MLP (16->1 conv, relu, 1->16 conv) on means and maxs, fused by a
  2->1 conv, bilinear-upsampled 4x4->8x8, sigmoid -> gate
  out = x * (1 + gate[b, h, y, x])

Implementation outline (per core: 32 batches, 4 chunks of 8 batches; SBUF
tile X [128, 8192] bf16 with partition p = b*16+h, free = c*64 + y*8 + x):

  means  : 16 ScalarE ACTIVATE(Copy, scale=1/512, accum_out) ops, one per
           4x4 grid cell, each summing the 512 (c, dy, dx) elements.
  maxs   : DVE tensor_max of y-pair halves (bf16 2x mode) then one
           reduce_max over (c, dx).
  MLP    : everything after the reductions is linear, so it folds into 3
           tiny TensorE matmuls with host-packed weights:
             hpreT[g,b] = sm[:, g]^T @ W1blk          (f1 on means|maxs)
             hcatT      = relu(hpreT + b1)            (ACT)
             qup[b,s]   = hcatT^T @ K2E               (wv-weighted upsample)
             gpre[p,s]  = W2blk^T @ qup               (f2 per head)
           gate2 = sigmoid(gpre + beta) + 1           (ACT x2)
  out    : X *= gate2 broadcast over c (DVE bf16 2x), DMA back.

All HBM traffic is bf16 (host converts f32 <-> bf16), halving the memory
time; tolerance is 2e-2 so ~0.3% bf16 error is fine.
"""

import sys

if "/opt/trn_rl_repo" not in sys.path:
    sys.path.insert(0, "/opt/trn_rl_repo")

import numpy as np

HEAD = 16
B, C, H, W = 256, 2048, 8, 8
NCORES = 8
BPC = B // NCORES          # 32 batches per core
CHUNK_B = 8                # batches per SBUF chunk (8*16 heads = 128 partitions)
NCHUNK = BPC // CHUNK_B    # 4
C16 = C // HEAD            # 128 channels per head group
SPAT = H * W               # 64
FREE = C16 * SPAT          # 8192 elems per partition per chunk

LAST_EXEC_NS = None        # filled when trace=True


def _upsample_matrix():
    """U[8,4]: bilinear 4->8, half-pixel centers (align_corners=False)."""
    U = np.zeros((8, 4), dtype=np.float64)
    for y in range(8):
        src = (y + 0.5) / 2.0 - 0.5
        i0 = int(np.floor(src))
        t = src - i0
        U[y, min(max(i0, 0), 3)] += 1.0 - t
        U[y, min(max(i0 + 1, 0), 3)] += t
    return U


def _pack_params(w1, b1, w2, b2, wv, bv):
    import ml_dtypes

    w1 = np.asarray(w1, np.float64).reshape(HEAD)
    w2 = np.asarray(w2, np.float64).reshape(HEAD)
    b2 = np.asarray(b2, np.float64).reshape(HEAD)
    wv = np.asarray(wv, np.float64).reshape(2)
    bv = float(np.asarray(bv, np.float64))
    b1 = float(np.asarray(b1, np.float64))

    p = np.arange(128)
    h16 = p % HEAD
    blk = p // HEAD  # which batch-slot this partition belongs to

    # fp32 consts [128, 11]: cols 0:8 W1blk (f1 weights, one col per batch
    # slot), col 8 betacol, col 9 b1 (relu bias), col 10 relu scale (1/512
    # for the two mean cells that arrive as raw sums from the DVE).
    cf32 = np.zeros((128, 11), np.float64)
    for b in range(CHUNK_B):
        cf32[:, b] = np.where(blk == b, w1[h16], 0.0)
    cf32[:, 8] = (wv[0] + wv[1]) * b2[h16] + bv
    cf32[:, 9] = b1
    cf32[:, 10] = 1.0

    U = _upsample_matrix()
    # K2[g, s] = U[y,i] * U[x,j], g = i*4+j, s = y*8+x
    K2 = np.einsum("yi,xj->ijyx", U, U).reshape(16, 64)

    # bf16 consts [128, 192]:
    #   rows 0:32, cols 0:64   K2E: wv0*K2 for the mean half, wv1*K2 for max
    #   rows 0:8,  cols 64:192 W2blk[b, p] = (p//16==b) * w2[p%16]
    cbf = np.zeros((128, 192), np.float64)
    cbf[0:16, 0:64] = wv[0] * K2
    cbf[16:32, 0:64] = wv[1] * K2
    for b in range(CHUNK_B):
        cbf[b, 64:192] = np.where(blk == b, w2[h16], 0.0)

    return (
        {
            "cf32": np.ascontiguousarray(cf32, np.float32),
            "cbf": np.ascontiguousarray(cbf.astype(np.float32), ml_dtypes.bfloat16),
        },
        b1,
    )


def _chain_input_dmas(nc, mybir):
    """Serialize the x-input DMAs at chunk granularity (chunk k waits for
    chunk k-1's second half). Without this all four chunks' input DMAs are
    queued immediately and share HBM bandwidth round-robin, so chunk 0's
    data — which gates all compute — arrives ~4x later than it needs to."""
    cum = {}
    xdmas = []
    for fn in nc.m.functions:
        for bb in fn.blocks:
            for ins in bb.instructions:
                si = getattr(ins, "sync_info", None)
                if si is None:
                    continue
                ups = list(si.on_update) if si.on_update else []
                for u in ups:
                    cum[u.id] = cum.get(u.id, 0) + u.update_value
                if (type(ins).__name__ == "InstDMACopy"
                        and str(getattr(ins.ins[0], "memref", "")) == "x"):
                    u = ups[0]
                    xdmas.append((ins, (u.id, u.ant_name, cum[u.id])))
    for k in range(1, len(xdmas) // 2):
        sem_id, name, val = xdmas[2 * k - 1][1]
        for idx in (2 * k, 2 * k + 1):
            ins = xdmas[idx][0]
            w = mybir.SyncWait(sync_type="semaphore", id=sem_id,
                              wait_mode="sem-ge-imm", wait_value=val,
                              ant_name=name)
            ins.sync_info.on_wait = list(ins.sync_info.on_wait or []) + [w]


def _split_multi_waits(nc, mybir):
    """Walrus codegen supports one sync-wait per instruction; hoist extras
    onto standalone InstEventSemaphore waits inserted right before, on the
    same engine (engines execute their stream in order, so this preserves
    the happens-before edges)."""
    n = 0
    for fn in nc.m.functions:
        for bb in fn.blocks:
            out = []
            for ins in bb.instructions:
                si = getattr(ins, "sync_info", None)
                waits = list(si.on_wait) if (si and si.on_wait) else []
                if len(waits) > 1:
                    for w in waits[:-1]:
                        n += 1
                        ev = mybir.InstEventSemaphore(
                            name=f"WSPLIT-{n}",
                            sync_info=mybir.SyncInfo(on_wait=[w], on_update=[]),
                        )
                        ev.engine = ins.engine
                        out.append(ev)
                    si.on_wait = [waits[-1]]
                out.append(ins)
            bb.instructions[:] = out


def _build(b1, split_waits=True):
    import concourse.bass as bass
    import concourse.tile as tile
    from concourse import mybir

    f32 = mybir.dt.float32
    bf16 = mybir.dt.bfloat16
    nc = bass.Bass()

    xd = nc.dram_tensor("x", [NCHUNK, 128, FREE], bf16, kind="ExternalInput")
    od = nc.dram_tensor("out", [NCHUNK, 128, FREE], bf16, kind="ExternalOutput")
    cf32d = nc.dram_tensor("cf32", [128, 11], f32, kind="ExternalInput")
    cbfd = nc.dram_tensor("cbf", [128, 192], bf16, kind="ExternalInput")

    AF = mybir.ActivationFunctionType

    with tile.TileContext(nc) as tc:
        with (
            tc.tile_pool(name="singles", bufs=1) as singles,
            tc.tile_pool(name="xin", bufs=4) as xpool,
            tc.tile_pool(name="mid", bufs=2) as mid,
            tc.tile_pool(name="small", bufs=2) as small,
            tc.tile_pool(name="wide", bufs=4) as wide,
            tc.tile_pool(name="psum", bufs=2, space="PSUM") as psum,
            tc.tile_pool(name="psum1", bufs=1, space="PSUM") as psum1,
        ):
            s_cf32 = singles.tile([128, 11], f32)
            nc.sync.dma_start(out=s_cf32, in_=cf32d[:, :])
            s_cbf = singles.tile([128, 192], bf16)
            nc.sync.dma_start(out=s_cbf, in_=cbfd[:, :])
            s_w1blk = s_cf32[:, 0:8]
            s_beta = s_cf32[:, 8:9]
            s_b1 = s_cf32[0:32, 9:10]
            s_rscale = s_cf32[0:32, 10:11]
            s_k2e = s_cbf[0:32, 0:64]
            s_w2blk = s_cbf[0:8, 64:192]
            # ACT writes of the accumulating sums land here (values unused)
            scratch = singles.tile([128, 512], bf16)

            # Walrus supports ONE sync-wait per instruction, so each const
            # DMA's semaphore must enter an engine's vector clock via a
            # dedicated absorber op before any real consumer (which also
            # carries a data wait) touches the tensor.
            d_a = singles.tile([128, 1], f32, tag="d_a")
            nc.scalar.copy(d_a, s_cf32[:, 0:1])          # ACT <- cf32 dma
            d_p = psum1.tile([1, 1], f32, tag="d_p")
            nc.tensor.matmul(d_p[:, :], s_cf32[0:1, 0:1], s_cf32[0:1, 0:1])
            d_p2 = psum1.tile([1, 1], f32, tag="d_p2")
            nc.tensor.matmul(d_p2[:, :], s_cbf[0:1, 0:1], s_cbf[0:1, 0:1])

            for ci in range(NCHUNK):
                # One X buffer per chunk (bufs=4): input DMAs start with no
                # waits and no WAR hazards ever form on X.
                X = xpool.tile([128, FREE], bf16, tag="X")
                nc.sync.dma_start(out=X[:, 0:FREE // 2], in_=xd[ci, :, 0:FREE // 2])
                nc.sync.dma_start(out=X[:, FREE // 2:FREE], in_=xd[ci, :, FREE // 2:FREE])
                Xf = X[:, :]

                # sm[:, 0:16] = per-grid-cell means (ScalarE accum);
                # sm[:, 16:32] = maxs (DVE). Disjoint column ranges, so the
                # two engines write concurrently; mm1's two waits get split
                # onto the idle PE's event-semaphores.
                sm = wide.tile([128, 32], f32, tag="sm")

                # Means pipeline: the DVE pre-folds the y-pairs of the 2x2
                # patches (unit-stride inner x -> bf16 2x mode) into
                # t1y[c i x], as two c-half ops so the first can start as
                # soon as the first input-DMA half lands. ScalarE then
                # accumulates all 16 grid cells (256 elems each).
                # (GpSimd is left idle on purpose: it shares an SBUF port
                # with the DVE, and any Pool work halves the throughput of
                # the DVE's 2-port bf16 modes.)
                X5 = Xf.rearrange("p (c i dy x) -> p c i dy x", c=C16, i=4, dy=2, x=8)
                t1y = mid.tile([128, 4096], bf16, tag="t1y")
                t1yv = t1y[:, :].rearrange("p (c i x) -> p c i x", c=C16, i=4, x=8)
                for ch in range(2):
                    cl = slice(ch * 64, (ch + 1) * 64)
                    nc.vector.tensor_add(
                        t1yv[:, cl, :, :],
                        X5[:, cl, :, 0, :], X5[:, cl, :, 1, :])
                # t1y free idx = c*32 + i*8 + (2j+dx)
                t1yc = t1y[:, :].rearrange(
                    "p (c i j dx) -> p c i j dx", c=C16, i=4, j=4, dx=2)
                if ci < NCHUNK - 1:
                    scr4 = scratch[:, 0:256].rearrange(
                        "p (c dx) -> p c dx", dx=2)
                    for g in range(16):
                        gi, gj = g // 4, g % 4
                        cell = t1yc[:, :, gi, gj, :]  # [p, c, dx]
                        nc.scalar.activation(
                            scr4,
                            cell,
                            AF.Copy,
                            bias=0.0,
                            scale=1.0 / 512.0,
                            accum_out=sm[:, g:g + 1],
                        )
                else:
                    # Last chunk: its cells would sit at the end of ScalarE's
                    # saturated queue and push the final mul+store far right.
                    # One DVE reduce lands in the DVE's otherwise-idle tail.
                    t1r5 = t1y[:, :].rearrange(
                        "p (c i j dx) -> p i j c dx", c=C16, i=4, j=4, dx=2)
                    mean16 = sm[:, 0:16].rearrange("p (i j) -> p i j", i=4)
                    nc.vector.reduce_sum(out=mean16, in_=t1r5,
                                         axis=mybir.AxisListType.XY)
                    nc.vector.tensor_scalar_mul(sm[:, 0:16], sm[:, 0:16],
                                                1.0 / 512.0)

                # max path: y-pair tensor_max (bf16 2x), then a c-halving
                # tensor_max tree (contiguous -> 2x) down to 16 channels,
                # then one small (c, dx) reduce_max.
                tm = mid.tile([128, 4096], bf16, tag="tm")
                tmv = tm[:, :].rearrange("p (c i x) -> p c i x", c=C16, i=4, x=8)
                nc.vector.tensor_max(tmv, X5[:, :, :, 0, :], X5[:, :, :, 1, :])
                n = 4096
                while n > 1024:
                    n //= 2
                    nc.vector.tensor_max(
                        tm[:, 0:n], tm[:, 0:n], tm[:, n:2 * n])
                # tm[:, 0:1024] = [c=32, i, x]; reduce over (c, dx)
                tmr = tm[:, 0:1024].rearrange(
                    "p (c i j dx) -> p i j c dx", c=32, i=4, j=4, dx=2)
                m16 = sm[:, 16:32].rearrange("p (i j) -> p i j", i=4)
                nc.vector.reduce_max(out=m16, in_=tmr, axis=mybir.AxisListType.XY)

                # MLP: 3 tiny matmuls + activations
                hpreT = psum.tile([32, 8], f32, tag="hpreT")
                nc.tensor.matmul(hpreT[:, :], sm[:, :], s_w1blk)

                hcatT = small.tile([32, 8], bf16, tag="hcatT")
                nc.scalar.activation(hcatT[:, :], hpreT[:, :], AF.Relu, bias=s_b1,
                                     scale=s_rscale)

                qup = psum.tile([8, 64], f32, tag="qup")
                nc.tensor.matmul(qup[:, :], hcatT[:, :], s_k2e)

                qupS = small.tile([8, 64], bf16, tag="qupS")
                nc.scalar.copy(qupS[:, :], qup[:, :])

                gpre = psum.tile([128, 64], f32, tag="gpre")
                nc.tensor.matmul(gpre[:, :], s_w2blk, qupS[:, :])

                gate = small.tile([128, 64], bf16, tag="gate")
                nc.scalar.activation(gate[:, :], gpre[:, :], AF.Sigmoid, bias=s_beta)
                gate2 = small.tile([128, 64], bf16, tag="gate2")
                nc.scalar.add(gate2[:, :], gate[:, :], 1.0)

                # out = gate2 * x, gate2 broadcast over the 128 group channels.
                # Last chunk: split mul+store in halves so the final store
                # overlaps the final multiply (shorter kernel tail).
                nhalf = 2 if ci == NCHUNK - 1 else 1
                cs = C16 // nhalf
                for hh in range(nhalf):
                    sl = slice(hh * cs * SPAT, (hh + 1) * cs * SPAT)
                    g_bc = gate2[:, :].unsqueeze(1).broadcast_to([128, cs, SPAT])
                    X3 = X[:, sl].rearrange("p (c s) -> p c s", s=SPAT)
                    nc.vector.tensor_mul(X3, g_bc, X3)
                    nc.sync.dma_start(out=od[ci, :, sl], in_=X[:, sl])

    if split_waits:
        _split_multi_waits(nc, mybir)
    return nc


def _shard_inputs(x, consts):
    import ml_dtypes

    xb = np.ascontiguousarray(x).astype(ml_dtypes.bfloat16)
    in_maps = []
    for i in range(NCORES):
        shard = xb[i * BPC:(i + 1) * BPC]  # [32, 2048, 8, 8]
        m = {"x": np.ascontiguousarray(shard.reshape(NCHUNK, 128, FREE))}
        m.update(consts)
        in_maps.append(m)
    return in_maps


def kernel(x, w1, b1, w2, b2, wv, bv, trace=False):
    global LAST_EXEC_NS
    from concourse.bass_utils import run_bass_kernel_spmd

    x = np.asarray(x, np.float32)
    consts, b1f = _pack_params(w1, b1, w2, b2, wv, bv)
    nc = _build(b1f)
    in_maps = _shard_inputs(x, consts)

    res = run_bass_kernel_spmd(nc, in_maps, core_ids=list(range(NCORES)),
                               trace=trace)
    LAST_EXEC_NS = res.exec_time_ns

    out = np.empty((B, C, H, W), np.float32)
    for i, r in enumerate(res.results):
        out[i * BPC:(i + 1) * BPC] = np.asarray(r["out"], np.float32).reshape(
            BPC, C, H, W)
    return out

